# revision 23
# baseline (speedup 1.0000x reference)
"""TRN2 Bass kernel for nn_AD_44006234915261 (dense transformer in-context RL model).

Data-parallel over batch: 8 batch items -> 8 NeuronCores, weights replicated.
Each core runs a 4-layer weight-shared post-norm transformer encoder
(L=512 tokens, D=1024, H=8 heads, DFF=4096) on one batch item and emits the
per-layer/per-head attention matrices plus the final hidden states.

Device layout: activations are kept feature-major ("xT": feature on the
partition axis, token on the free axis) so every weight matmul is a plain
lhsT(=weight chunk) x rhs(=activation) PE op. Per-token reductions
(LayerNorm stats, softmax denominators) are done with all-ones stationary
matmuls, which produce the reduction broadcast across partitions for free.
All matmuls run in fp16 (full PE rate, pipelined weight loads) with fp32
PSUM accumulation; softmax/LayerNorm statistics are computed in fp32.
Softmax is computed without max-subtraction (scores are bounded ~|2.5|),
1/sum via exp(-ln(sum)) on ScalarE (alternating with VectorE reciprocal),
and rstd via exp(-0.5*ln(var+eps)) so everything stays in one ACT table
set (exp/ln), avoiding table-load thrash.

Host side: input embedding (tiny), the prediction head / loss / accuracy
(tiny), and assembly of the attention output (transpose per head).
"""

import math
import os

import numpy as np

N_LAYERS = 4
H = 8
D = 1024
DFF = 4096
L = 512
N_CTX = 511
HD = D // H
GRID = 9
LN_EPS = 1e-5
LABEL_SMOOTH = 0.1
NC = 8  # cores = batch
DC = D // 128      # 8 feature chunks
FC = DFF // 128    # 32 hidden chunks
LC = L // 128      # 4 token chunks

_CACHE = {}

# set by kernel() when TRN_KERNEL_TRACE=1; read by test.py
last_exec_time_ns = None


def _build_nc(n_layers, skips=(), debug=False):
    import concourse.bass as bass
    import concourse.tile as tile
    from concourse import bacc, mybir

    F32 = mybir.dt.float32
    F16 = mybir.dt.float16
    AF = mybir.ActivationFunctionType
    OP = mybir.AluOpType

    class _Bacc(bacc.Bacc):
        # Route Exp and Ln to the one table set containing both
        # (natural_log_exp_and_others) so LayerNorm/softmax chains don't
        # thrash ACT table loads between exp_and_others and natural_log.
        def insert_act_table_loads(self):
            import bass_rust as _br
            from concourse.hw_specs import get_activation_tables
            has_activation = any(
                isinstance(i, mybir.InstActivation)
                for b in self.main_func.blocks
                for i in b.instructions
            )
            if not has_activation:
                return
            tables = []
            for name, fns in get_activation_tables(self.m.arch).items():
                if name == "exp_and_others":
                    fns = fns - {AF.Exp}
                if name == "natural_log":
                    fns = fns - {AF.Ln}
                tables.append((name, fns))
            _br.insert_act_table_loads(self, tables)

    nc = _Bacc()
    x_in = nc.declare_dram_parameter("x0T", [128, DC, L], F16, isOutput=False)
    wqk_in = nc.declare_dram_parameter("wqk", [2 * H, 128, DC, 128], F16, isOutput=False)
    wv_in = nc.declare_dram_parameter("wv", [DC, 128, D], F16, isOutput=False)
    wo_in = nc.declare_dram_parameter("wo", [DC, 128, DC, 128], F16, isOutput=False)
    w1_in = nc.declare_dram_parameter("w1", [FC, 128, DC, 128], F16, isOutput=False)
    w2_in = nc.declare_dram_parameter("w2", [DC, 128, FC, 128], F16, isOutput=False)
    bqk_in = nc.declare_dram_parameter("bqk", [128, 2 * H], F32, isOutput=False)
    bo_in = nc.declare_dram_parameter("bo", [128, DC], F32, isOutput=False)
    b1_in = nc.declare_dram_parameter("b1", [128, FC], F32, isOutput=False)
    b2_in = nc.declare_dram_parameter("b2", [128, DC], F32, isOutput=False)
    attn_out = nc.declare_dram_parameter(
        "attnT", [n_layers, H, 128, LC, L], F16, isOutput=True)
    if debug:
        qk_dbg = nc.declare_dram_parameter("qk_dbg", [128, 2 * H, L], F32, isOutput=True)
        v_dbg = nc.declare_dram_parameter("v_dbg", [128, LC, D], F32, isOutput=True)
        ctx_dbg = nc.declare_dram_parameter("ctx_dbg", [128, DC, L], F32, isOutput=True)
    x_out = nc.declare_dram_parameter("xoutT", [128, DC, L], F32, isOutput=True)

    inv_sqrt_hd = float(1.0 / math.sqrt(HD))

    with tile.TileContext(nc) as tc:
        with tc.tile_pool(name="glob", bufs=1) as glob, \
             tc.tile_pool(name="wsmall", bufs=3) as wsmall, \
             tc.tile_pool(name="wbig", bufs=2) as wbig, \
             tc.tile_pool(name="wv_p", bufs=DC) as wv_p, \
             tc.tile_pool(name="big", bufs=1) as big, \
             tc.tile_pool(name="v_p", bufs=1) as v_p, \
             tc.tile_pool(name="ctx_p", bufs=1) as ctx_p, \
             tc.tile_pool(name="e_p", bufs=4) as e_p, \
             tc.tile_pool(name="r_p", bufs=4) as r_p, \
             tc.tile_pool(name="a_p", bufs=4) as a_p, \
             tc.tile_pool(name="stat", bufs=3) as stat, \
             tc.tile_pool(name="sq_p", bufs=2) as sq_p, \
             tc.tile_pool(name="ps_mm", bufs=2, space="PSUM") as ps_mm, \
             tc.tile_pool(name="ps_sc", bufs=2, space="PSUM") as ps_sc, \
             tc.tile_pool(name="ps_acc", bufs=2, space="PSUM") as ps_acc:

            xT = glob.tile([128, DC, L], F16)
            ones_f = glob.tile([128, 128], F32)
            onesr = glob.tile([128, 128], F16)
            bqk_t = glob.tile([128, 2 * H], F32)
            bo_t = glob.tile([128, DC], F32)
            b1_t = glob.tile([128, FC], F32)
            b2_t = glob.tile([128, DC], F32)
            eps_t = glob.tile([128, 1], F32)
            nc.sync.dma_start(out=xT, in_=x_in[:])
            nc.vector.memset(ones_f, 1.0)
            nc.vector.tensor_copy(onesr, ones_f)
            nc.vector.memset(eps_t, LN_EPS)
            nc.sync.dma_start(out=bqk_t, in_=bqk_in[:])
            nc.sync.dma_start(out=bo_t, in_=bo_in[:])
            nc.sync.dma_start(out=b1_t, in_=b1_in[:])
            nc.sync.dma_start(out=b2_t, in_=b2_in[:])

            def ln_stats_start():
                # two PSUM accumulators: sum(x), sum(x^2); fed per-chunk
                ps_sx = ps_acc.tile([128, L], F32, tag="acc")
                ps_sq = ps_acc.tile([128, L], F32, tag="acc")
                return ps_sx, ps_sq

            def ln_stats_emit(st2, j):
                ps_sx, ps_sq = st2
                nc.tensor.matmul(ps_sx, onesr, xT[:, j, :],
                                 start=(j == 0), stop=(j == DC - 1))
                sq = sq_p.tile([128, L], F16, tag="sq")
                nc.vector.tensor_mul(sq, xT[:, j, :], xT[:, j, :])
                nc.tensor.matmul(ps_sq, onesr, sq,
                                 start=(j == 0), stop=(j == DC - 1))

            def ln_rstd(st2):
                ps_sx, ps_sq = st2
                mean_t = stat.tile([128, L], F32, tag="stat")
                nc.vector.tensor_scalar_mul(mean_t, ps_sx, 1.0 / D)
                m2 = stat.tile([128, L], F32, tag="stat")
                nc.vector.tensor_mul(m2, mean_t, mean_t)
                var = stat.tile([128, L], F32, tag="stat")
                nc.vector.scalar_tensor_tensor(
                    var, ps_sq, 1.0 / D, m2, op0=OP.mult, op1=OP.subtract)
                lnv = stat.tile([128, L], F32, tag="stat")
                nc.scalar.activation(lnv, var, AF.Ln, bias=eps_t[:])
                rstd = stat.tile([128, L], F16, tag="stat16")
                nc.scalar.activation(rstd, lnv, AF.Exp, scale=-0.5)
                mr = stat.tile([128, L], F16, tag="stat16")
                nc.vector.tensor_mul(mr, mean_t, rstd)
                return rstd, mr

            def ln_apply(rm, j):
                rstd, mr = rm
                tmp = stat.tile([128, L], F16, tag="tmp")
                nc.vector.tensor_mul(tmp, xT[:, j, :], rstd)
                nc.vector.tensor_sub(xT[:, j, :], tmp, mr)

            for lay in range(n_layers):
                # ---- QKV projections (applies previous layer's LN2) ----
                qkT = big.tile([128, 2 * H, L], F16, tag="bigact")
                for j in range(2 * H):
                    wt = wsmall.tile([128, DC, 128], F16, tag="w")
                    nc.sync.dma_start(out=wt, in_=wqk_in[j])
                    ps = ps_mm.tile([128, L], F32, tag="mm")
                    for d in range(DC):
                        nc.tensor.matmul(ps, wt[:, d, :], xT[:, d, :],
                                         start=(d == 0), stop=(d == DC - 1))
                    nc.vector.tensor_scalar_add(qkT[:, j, :], ps, bqk_t[:, j:j + 1])
                V = v_p.tile([128, LC, D], F16, tag="v")
                for hh in range(2):
                    wv_tiles = []
                    for d in range(DC):
                        wvt = wv_p.tile([128, 512], F16, tag="wv")
                        nc.sync.dma_start(
                            out=wvt, in_=wv_in[d, :, 512 * hh:512 * (hh + 1)])
                        wv_tiles.append(wvt)
                    for i in range(LC):
                        ps = ps_mm.tile([128, L], F32, tag="mm")
                        for d in range(DC):
                            nc.tensor.matmul(
                                ps, xT[:, d, 128 * i:128 * (i + 1)], wv_tiles[d],
                                start=(d == 0), stop=(d == DC - 1))
                        nc.vector.tensor_copy(V[:, i, 512 * hh:512 * (hh + 1)], ps)

                if debug and lay == 0:
                    for c in range(2 * H):
                        st = a_p.tile([128, L], F32, tag="attn")
                        nc.vector.tensor_copy(st, qkT[:, c, :])
                        nc.sync.dma_start(out=qk_dbg[:, c, :], in_=st)
                    for c in range(LC):
                        for hh2 in range(2):
                            st = a_p.tile([128, L], F32, tag="attn")
                            nc.vector.tensor_copy(st, V[:, c, 512 * hh2:512 * (hh2 + 1)])
                            nc.sync.dma_start(
                                out=v_dbg[:, c, 512 * hh2:512 * (hh2 + 1)], in_=st)
                # ---- attention heads ----
                ctxT = ctx_p.tile([128, DC, L], F16, tag="ctx")
                for head in range(H):
                    qs = qkT[:, head, :]
                    ks = qkT[:, H + head, :]
                    eTs = []
                    for half in range(2):
                        ps_s = ps_sc.tile([128, 2, L], F32, tag="score")
                        for ii in range(2):
                            i = 2 * half + ii
                            nc.tensor.matmul(ps_s[:, ii, :],
                                             ks[:, 128 * i:128 * (i + 1)], qs,
                                             start=True, stop=True)
                        eT = e_p.tile([128, 2, L], F16, tag="e")
                        nc.scalar.activation(eT, ps_s, AF.Exp, scale=inv_sqrt_hd)
                        eTs.append(eT)
                    es = lambda i: eTs[i // 2][:, i % 2, :]
                    ps_d = ps_acc.tile([128, L], F32, tag="acc")
                    for i in range(LC):
                        nc.tensor.matmul(ps_d, onesr, es(i),
                                         start=(i == 0), stop=(i == LC - 1))
                    if head % 2 == 0:
                        lns = r_p.tile([128, L], F32, tag="lns")
                        nc.scalar.activation(lns, ps_d, AF.Ln)
                        rbc = r_p.tile([128, L], F16, tag="r")
                        nc.scalar.activation(rbc, lns, AF.Exp, scale=-1.0)
                    else:
                        rbc32 = r_p.tile([128, L], F32, tag="lns")
                        nc.vector.reciprocal(rbc32, ps_d)
                        rbc = r_p.tile([128, L], F16, tag="r")
                        nc.vector.tensor_copy(rbc, rbc32)
                    ps_c = ps_acc.tile([128, L], F32, tag="acc")
                    for i in range(LC):
                        nc.tensor.matmul(
                            ps_c, V[:, i, 128 * head:128 * (head + 1)], es(i),
                            start=(i == 0), stop=(i == LC - 1))
                    nc.vector.tensor_mul(ctxT[:, head, :], ps_c, rbc)
                    if "attn_dma" not in skips:
                        at = a_p.tile([128, LC, L], F16, tag="attn")
                        for i in range(LC):
                            nc.gpsimd.tensor_mul(at[:, i, :], es(i), rbc)
                        nc.sync.dma_start(out=attn_out[lay, head], in_=at)

                # ---- out_proj + residual (LN1 stats interleaved) ----
                st2 = ln_stats_start()
                for j in range(DC):
                    wt = wsmall.tile([128, DC, 128], F16, tag="w")
                    nc.sync.dma_start(out=wt, in_=wo_in[j])
                    ps = ps_mm.tile([128, L], F32, tag="mm")
                    for d in range(DC):
                        nc.tensor.matmul(ps, wt[:, d, :], ctxT[:, d, :],
                                         start=(d == 0), stop=(d == DC - 1))
                    nc.vector.scalar_tensor_tensor(
                        xT[:, j, :], ps, bo_t[:, j:j + 1], xT[:, j, :],
                        op0=OP.add, op1=OP.add)
                    ln_stats_emit(st2, j)
                if "ln1" not in skips:
                    rm1 = ln_rstd(st2)
                    for d in range(DC):
                        ln_apply(rm1, d)

                # ---- FFN ----
                if "ffn" in skips:
                    continue
                hT = big.tile([128, FC, L], F16, tag="bigact")
                for j in range(FC):
                    wt = wsmall.tile([128, DC, 128], F16, tag="w")
                    nc.sync.dma_start(out=wt, in_=w1_in[j])
                    ps = ps_mm.tile([128, L], F32, tag="mm")
                    for d in range(DC):
                        nc.tensor.matmul(ps, wt[:, d, :], xT[:, d, :],
                                         start=(d == 0), stop=(d == DC - 1))
                    nc.scalar.activation(hT[:, j, :], ps, AF.Gelu,
                                         bias=b1_t[:, j:j + 1])
                st2 = ln_stats_start()
                for j in range(DC):
                    wt = wbig.tile([128, FC, 128], F16, tag="w2")
                    nc.sync.dma_start(out=wt, in_=w2_in[j])
                    ps = ps_mm.tile([128, L], F32, tag="mm")
                    for f in range(FC):
                        nc.tensor.matmul(ps, wt[:, f, :], hT[:, f, :],
                                         start=(f == 0), stop=(f == FC - 1))
                    nc.vector.scalar_tensor_tensor(
                        xT[:, j, :], ps, b2_t[:, j:j + 1], xT[:, j, :],
                        op0=OP.add, op1=OP.add)
                    ln_stats_emit(st2, j)
                if "ln2" not in skips:
                    rm2 = ln_rstd(st2)
                    for d in range(DC):
                        ln_apply(rm2, d)

            xout_stage = glob.tile([128, DC, L], F32)
            nc.vector.tensor_copy(xout_stage, xT)
            nc.sync.dma_start(out=x_out[:], in_=xout_stage)
    nc.finalize()
    return nc


def _get_nc(n_layers):
    skips = tuple(s for s in os.environ.get("ATT_SKIPS", "").split(",") if s)
    debug = os.environ.get("ATT_DEBUG", "0") == "1"
    key = (n_layers, skips, debug)
    if key not in _CACHE:
        _CACHE[key] = _build_nc(n_layers, skips, debug)
    return _CACHE[key]


def _block4(w, rows, cols):
    # (rows*128, cols*128) weight -> [j, p, d, m] with
    # block[j, p, d, m] = w[j*128+m, d*128+p], laid out to match the SBUF
    # tile (128 partitions, cols chunks, 128) so the DMA is a plain copy.
    return np.ascontiguousarray(
        w.reshape(rows, 128, cols, 128).transpose(0, 3, 2, 1))


def kernel(states, actions, next_states, rewards, pos_embedding,
           embed_context_w, embed_context_b, embed_query_table,
           in_proj_w, in_proj_b, out_proj_w, out_proj_b,
           linear1_w, linear1_b, linear2_w, linear2_b,
           norm1_g, norm1_b, norm2_g, norm2_b, pred_w, pred_b,
           query_states, target_actions):
    global last_exec_time_ns
    from concourse.bass_utils import run_bass_kernel_spmd

    f32 = np.float32
    states = np.asarray(states, f32)
    actions = np.asarray(actions, f32)
    next_states = np.asarray(next_states, f32)
    rewards = np.asarray(rewards, f32)
    pos_embedding = np.asarray(pos_embedding, f32)
    in_proj_w = np.asarray(in_proj_w, f32)
    out_proj_w = np.asarray(out_proj_w, f32)
    linear1_w = np.asarray(linear1_w, f32)
    linear2_w = np.asarray(linear2_w, f32)

    # structural assumptions baked into the device graph
    assert np.all(np.asarray(in_proj_b)[2 * D:] == 0), "V bias unsupported"
    for g in (norm1_g, norm2_g):
        assert np.all(np.asarray(g) == 1), "LN gain unsupported"
    for b in (norm1_b, norm2_b):
        assert np.all(np.asarray(b) == 0), "LN bias unsupported"

    B = states.shape[0]
    assert B == NC

    # ---- host: input embedding ----
    qidx = np.asarray(query_states)[:, 0] * GRID + np.asarray(query_states)[:, 1]
    q_embed = np.asarray(embed_query_table, f32)[qidx][:, None, :]
    context = np.concatenate(
        [states, actions, rewards[..., None], next_states], axis=-1)
    ctx_embed = context @ np.asarray(embed_context_w, f32).T \
        + np.asarray(embed_context_b, f32)
    x0 = np.concatenate([ctx_embed, q_embed], axis=1) + pos_embedding[:, :L]

    # ---- shard + weight prep ----
    f16 = np.float16
    wqk = _block4(in_proj_w[:2 * D], 16, DC).astype(f16)
    wv = np.ascontiguousarray(in_proj_w[2 * D:].T.reshape(DC, 128, D)).astype(f16)
    wo = _block4(out_proj_w, DC, DC).astype(f16)
    w1 = _block4(linear1_w, FC, DC).astype(f16)
    w2 = _block4(linear2_w, DC, FC).astype(f16)
    bqk = np.ascontiguousarray(np.asarray(in_proj_b, f32)[:2 * D].reshape(16, 128).T)
    bo = np.ascontiguousarray(np.asarray(out_proj_b, f32).reshape(DC, 128).T)
    b1 = np.ascontiguousarray(np.asarray(linear1_b, f32).reshape(FC, 128).T)
    b2 = np.ascontiguousarray(np.asarray(linear2_b, f32).reshape(DC, 128).T)
    shared = dict(wqk=wqk, wv=wv, wo=wo, w1=w1, w2=w2,
                  bqk=bqk, bo=bo, b1=b1, b2=b2)
    in_maps = []
    for b in range(B):
        x0T = np.ascontiguousarray(
            x0[b].T.reshape(DC, 128, L).transpose(1, 0, 2)).astype(np.float16)
        in_maps.append(dict(x0T=x0T, **shared))

    n_layers = int(os.environ.get("ATT_LAYERS", N_LAYERS))
    nc = _get_nc(n_layers)
    trace = os.environ.get("TRN_KERNEL_TRACE", "0") == "1"
    res = run_bass_kernel_spmd(nc, in_maps, core_ids=list(range(NC)), trace=trace)
    last_exec_time_ns = res.exec_time_ns
    globals()["last_results"] = res

    # ---- host: gather + head ----
    attns = np.empty((n_layers, B, H, L, L), f32)
    xlast = np.empty((B, D), f32)
    for b in range(B):
        r = res.results[b]
        # attnT[lay, head, m, i, l]: token m_glob = i*128+m -> [lay, head, l, m_glob]
        a = r["attnT"].astype(f32)
        attns[:, b] = a.transpose(0, 1, 4, 3, 2).reshape(n_layers, H, L, L)
        xT_full = r["xoutT"].transpose(1, 0, 2).reshape(D, L)
        xlast[b] = xT_full[:, L - 1]

    logits = xlast @ np.asarray(pred_w, f32).T + np.asarray(pred_b, f32)
    mx = logits.max(-1, keepdims=True)
    lse = np.log(np.exp(logits - mx).sum(-1, keepdims=True)) + mx
    logp = logits - lse
    tgt = np.asarray(target_actions).astype(np.int64)
    nll = -np.take_along_axis(logp, tgt[:, None], axis=-1)[:, 0]
    loss = np.mean((1.0 - LABEL_SMOOTH) * nll - LABEL_SMOOTH * logp.mean(-1))
    acc = np.mean((np.argmax(logits, -1) == tgt).astype(f32))
    return np.float32(loss), np.float32(acc), attns


# revision 25
# speedup vs baseline: 1.2363x; 1.2363x over previous
"""TRN2 Bass kernel for nn_AD_44006234915261 (dense transformer in-context RL model).

Data-parallel over batch: 8 batch items -> 8 NeuronCores, weights replicated.
Each core runs a 4-layer weight-shared post-norm transformer encoder
(L=512 tokens, D=1024, H=8 heads, DFF=4096) on one batch item and emits the
per-layer/per-head attention matrices plus the final hidden states.

Device layout: activations are kept feature-major ("xT": feature on the
partition axis, token on the free axis) so every weight matmul is a plain
lhsT(=weight chunk) x rhs(=activation) PE op. Per-token reductions
(LayerNorm stats, softmax denominators) are done with all-ones stationary
matmuls, which produce the reduction broadcast across partitions for free.
All matmuls run in fp16 (full PE rate, pipelined weight loads) with fp32
PSUM accumulation; softmax/LayerNorm statistics are computed in fp32.
Softmax is computed without max-subtraction (scores are bounded ~|2.5|),
1/sum via exp(-ln(sum)) on ScalarE (alternating with VectorE reciprocal),
and rstd via exp(-0.5*ln(var+eps)) so everything stays in one ACT table
set (exp/ln), avoiding table-load thrash.

Host side: input embedding (tiny), the prediction head / loss / accuracy
(tiny), and assembly of the attention output (transpose per head).
"""

import math
import os

import numpy as np

N_LAYERS = 4
H = 8
D = 1024
DFF = 4096
L = 512
N_CTX = 511
HD = D // H
GRID = 9
LN_EPS = 1e-5
LABEL_SMOOTH = 0.1
NC = 8  # cores = batch
DC = D // 128      # 8 feature chunks
FC = DFF // 128    # 32 hidden chunks
LC = L // 128      # 4 token chunks

_CACHE = {}

# set by kernel() when TRN_KERNEL_TRACE=1; read by test.py
last_exec_time_ns = None


def _build_nc(n_layers, skips=(), debug=False):
    import concourse.bass as bass
    import concourse.tile as tile
    from concourse import bacc, mybir

    F32 = mybir.dt.float32
    F16 = mybir.dt.float16
    AF = mybir.ActivationFunctionType
    OP = mybir.AluOpType

    class _Bacc(bacc.Bacc):
        # Route Exp and Ln to the one table set containing both
        # (natural_log_exp_and_others) so LayerNorm/softmax chains don't
        # thrash ACT table loads between exp_and_others and natural_log.
        def insert_act_table_loads(self):
            import bass_rust as _br
            from concourse.hw_specs import get_activation_tables
            has_activation = any(
                isinstance(i, mybir.InstActivation)
                for b in self.main_func.blocks
                for i in b.instructions
            )
            if not has_activation:
                return
            tables = []
            for name, fns in get_activation_tables(self.m.arch).items():
                if name == "exp_and_others":
                    fns = fns - {AF.Exp}
                if name == "natural_log":
                    fns = fns - {AF.Ln}
                tables.append((name, fns))
            _br.insert_act_table_loads(self, tables)

    nc = _Bacc()
    x_in = nc.declare_dram_parameter("x0T", [128, DC, L], F16, isOutput=False)
    wqk_in = nc.declare_dram_parameter("wqk", [2 * H, 128, DC, 128], F16, isOutput=False)
    wv_in = nc.declare_dram_parameter("wv", [DC, 128, D], F16, isOutput=False)
    wo_in = nc.declare_dram_parameter("wo", [DC, 128, DC, 128], F16, isOutput=False)
    w1_in = nc.declare_dram_parameter("w1", [FC, 128, DC, 128], F16, isOutput=False)
    w2_in = nc.declare_dram_parameter("w2", [DC, 128, FC, 128], F16, isOutput=False)
    bqk_in = nc.declare_dram_parameter("bqk", [128, 2 * H], F32, isOutput=False)
    bo_in = nc.declare_dram_parameter("bo", [128, DC], F32, isOutput=False)
    b1_in = nc.declare_dram_parameter("b1", [128, FC], F32, isOutput=False)
    b2_in = nc.declare_dram_parameter("b2", [128, DC], F32, isOutput=False)
    attn_out = nc.declare_dram_parameter(
        "attnT", [n_layers, H, 128, LC, L], F16, isOutput=True)
    if debug:
        qk_dbg = nc.declare_dram_parameter("qk_dbg", [128, 2 * H, L], F32, isOutput=True)
        v_dbg = nc.declare_dram_parameter("v_dbg", [128, LC, D], F32, isOutput=True)
        ctx_dbg = nc.declare_dram_parameter("ctx_dbg", [128, DC, L], F32, isOutput=True)
    x_out = nc.declare_dram_parameter("xoutT", [128, DC, L], F32, isOutput=True)

    inv_sqrt_hd = float(1.0 / math.sqrt(HD))

    with tile.TileContext(nc) as tc:
        with tc.tile_pool(name="glob", bufs=1) as glob, \
             tc.tile_pool(name="wsmall", bufs=4) as wsmall, \
             tc.tile_pool(name="wbig", bufs=2) as wbig, \
             tc.tile_pool(name="wv_p", bufs=DC) as wv_p, \
             tc.tile_pool(name="big", bufs=1) as big, \
             tc.tile_pool(name="v_p", bufs=1) as v_p, \
             tc.tile_pool(name="ctx_p", bufs=1) as ctx_p, \
             tc.tile_pool(name="e_p", bufs=6) as e_p, \
             tc.tile_pool(name="r_p", bufs=4) as r_p, \
             tc.tile_pool(name="a_p", bufs=4) as a_p, \
             tc.tile_pool(name="stat", bufs=3) as stat, \
             tc.tile_pool(name="sq_p", bufs=2) as sq_p, \
             tc.tile_pool(name="ps_mm", bufs=2, space="PSUM") as ps_mm, \
             tc.tile_pool(name="ps_sc", bufs=2, space="PSUM") as ps_sc, \
             tc.tile_pool(name="ps_acc", bufs=2, space="PSUM") as ps_acc:

            xT = glob.tile([128, DC, L], F16)
            ones_f = glob.tile([128, 128], F32)
            onesr = glob.tile([128, 128], F16)
            bqk_t = glob.tile([128, 2 * H], F32)
            bo_t = glob.tile([128, DC], F32)
            b1_t = glob.tile([128, FC], F32)
            b2_t = glob.tile([128, DC], F32)
            eps_t = glob.tile([128, 1], F32)
            nc.sync.dma_start(out=xT, in_=x_in[:])
            nc.vector.memset(ones_f, 1.0)
            nc.vector.tensor_copy(onesr, ones_f)
            nc.vector.memset(eps_t, LN_EPS)
            nc.sync.dma_start(out=bqk_t, in_=bqk_in[:])
            nc.sync.dma_start(out=bo_t, in_=bo_in[:])
            nc.sync.dma_start(out=b1_t, in_=b1_in[:])
            nc.sync.dma_start(out=b2_t, in_=b2_in[:])

            def ln_stats_start():
                # two PSUM accumulators: sum(x), sum(x^2); fed per-chunk
                ps_sx = ps_acc.tile([128, L], F32, tag="acc")
                ps_sq = ps_acc.tile([128, L], F32, tag="acc")
                return ps_sx, ps_sq

            def ln_stats_emit(st2, j):
                ps_sx, ps_sq = st2
                nc.tensor.matmul(ps_sx, onesr, xT[:, j, :],
                                 start=(j == 0), stop=(j == DC - 1))
                sq = sq_p.tile([128, L], F16, tag="sq")
                nc.vector.tensor_mul(sq, xT[:, j, :], xT[:, j, :])
                nc.tensor.matmul(ps_sq, onesr, sq,
                                 start=(j == 0), stop=(j == DC - 1))

            def ln_rstd(st2):
                ps_sx, ps_sq = st2
                mean_t = stat.tile([128, L], F32, tag="stat")
                nc.vector.tensor_scalar_mul(mean_t, ps_sx, 1.0 / D)
                m2 = stat.tile([128, L], F32, tag="stat")
                nc.vector.tensor_mul(m2, mean_t, mean_t)
                var = stat.tile([128, L], F32, tag="stat")
                nc.vector.scalar_tensor_tensor(
                    var, ps_sq, 1.0 / D, m2, op0=OP.mult, op1=OP.subtract)
                lnv = stat.tile([128, L], F32, tag="stat")
                nc.scalar.activation(lnv, var, AF.Ln, bias=eps_t[:])
                rstd = stat.tile([128, L], F16, tag="stat16")
                nc.scalar.activation(rstd, lnv, AF.Exp, scale=-0.5)
                mr = stat.tile([128, L], F16, tag="stat16")
                nc.vector.tensor_mul(mr, mean_t, rstd)
                return rstd, mr

            def ln_apply(rm, j):
                rstd, mr = rm
                tmp = stat.tile([128, L], F16, tag="tmp")
                nc.vector.tensor_mul(tmp, xT[:, j, :], rstd)
                nc.vector.tensor_sub(xT[:, j, :], tmp, mr)

            for lay in range(n_layers):
                # ---- QKV projections (applies previous layer's LN2) ----
                qkT = big.tile([128, 2 * H, L], F16, tag="bigact")
                for j in range(2 * H):
                    wt = wsmall.tile([128, DC, 128], F16, tag="w")
                    nc.sync.dma_start(out=wt, in_=wqk_in[j])
                    ps = ps_mm.tile([128, L], F32, tag="mm")
                    for d in range(DC):
                        nc.tensor.matmul(ps, wt[:, d, :], xT[:, d, :],
                                         start=(d == 0), stop=(d == DC - 1))
                    nc.vector.tensor_scalar_add(qkT[:, j, :], ps, bqk_t[:, j:j + 1])
                V = v_p.tile([128, LC, D], F16, tag="v")
                for hh in range(2):
                    wv_tiles = []
                    for d in range(DC):
                        wvt = wv_p.tile([128, 512], F16, tag="wv")
                        nc.sync.dma_start(
                            out=wvt, in_=wv_in[d, :, 512 * hh:512 * (hh + 1)])
                        wv_tiles.append(wvt)
                    for i in range(LC):
                        ps = ps_mm.tile([128, L], F32, tag="mm")
                        for d in range(DC):
                            nc.tensor.matmul(
                                ps, xT[:, d, 128 * i:128 * (i + 1)], wv_tiles[d],
                                start=(d == 0), stop=(d == DC - 1))
                        nc.vector.tensor_copy(V[:, i, 512 * hh:512 * (hh + 1)], ps)

                if debug and lay == 0:
                    for c in range(2 * H):
                        st = a_p.tile([128, L], F32, tag="attn")
                        nc.vector.tensor_copy(st, qkT[:, c, :])
                        nc.sync.dma_start(out=qk_dbg[:, c, :], in_=st)
                    for c in range(LC):
                        for hh2 in range(2):
                            st = a_p.tile([128, L], F32, tag="attn")
                            nc.vector.tensor_copy(st, V[:, c, 512 * hh2:512 * (hh2 + 1)])
                            nc.sync.dma_start(
                                out=v_dbg[:, c, 512 * hh2:512 * (hh2 + 1)], in_=st)
                # ---- attention heads ----
                ctxT = ctx_p.tile([128, DC, L], F16, tag="ctx")
                for head in range(H):
                    qs = qkT[:, head, :]
                    ks = qkT[:, H + head, :]
                    eTs = []
                    for half in range(2):
                        ps_s = ps_sc.tile([128, 2, L], F32, tag="score")
                        for ii in range(2):
                            i = 2 * half + ii
                            nc.tensor.matmul(ps_s[:, ii, :],
                                             ks[:, 128 * i:128 * (i + 1)], qs,
                                             start=True, stop=True)
                        eT = e_p.tile([128, 2, L], F16, tag="e")
                        nc.scalar.activation(eT, ps_s, AF.Exp, scale=inv_sqrt_hd)
                        eTs.append(eT)
                    es = lambda i: eTs[i // 2][:, i % 2, :]
                    ps_d = ps_acc.tile([128, L], F32, tag="acc")
                    for i in range(LC):
                        nc.tensor.matmul(ps_d, onesr, es(i),
                                         start=(i == 0), stop=(i == LC - 1))
                    if head % 2 == 0:
                        lns = r_p.tile([128, L], F32, tag="lns")
                        nc.scalar.activation(lns, ps_d, AF.Ln)
                        rbc = r_p.tile([128, L], F16, tag="r")
                        nc.scalar.activation(rbc, lns, AF.Exp, scale=-1.0)
                    else:
                        rbc32 = r_p.tile([128, L], F32, tag="lns")
                        nc.vector.reciprocal(rbc32, ps_d)
                        rbc = r_p.tile([128, L], F16, tag="r")
                        nc.vector.tensor_copy(rbc, rbc32)
                    ps_c = ps_acc.tile([128, L], F32, tag="acc")
                    for i in range(LC):
                        nc.tensor.matmul(
                            ps_c, V[:, i, 128 * head:128 * (head + 1)], es(i),
                            start=(i == 0), stop=(i == LC - 1))
                    nc.vector.tensor_mul(ctxT[:, head, :], ps_c, rbc)
                    if "attn_dma" not in skips:
                        at = a_p.tile([128, LC, L], F16, tag="attn")
                        for i in range(LC):
                            eng = nc.gpsimd if i % 2 == 0 else nc.vector
                            eng.tensor_mul(at[:, i, :], es(i), rbc)
                        nc.sync.dma_start(out=attn_out[lay, head], in_=at)

                # ---- out_proj + residual (LN1 stats interleaved) ----
                st2 = ln_stats_start()
                for j in range(DC):
                    wt = wsmall.tile([128, DC, 128], F16, tag="w")
                    nc.sync.dma_start(out=wt, in_=wo_in[j])
                    ps = ps_mm.tile([128, L], F32, tag="mm")
                    for d in range(DC):
                        nc.tensor.matmul(ps, wt[:, d, :], ctxT[:, d, :],
                                         start=(d == 0), stop=(d == DC - 1))
                    nc.vector.scalar_tensor_tensor(
                        xT[:, j, :], ps, bo_t[:, j:j + 1], xT[:, j, :],
                        op0=OP.add, op1=OP.add)
                    ln_stats_emit(st2, j)
                if "ln1" not in skips:
                    rm1 = ln_rstd(st2)
                    for d in range(DC):
                        ln_apply(rm1, d)

                # ---- FFN ----
                if "ffn" in skips:
                    continue
                hT = big.tile([128, FC, L], F16, tag="bigact")
                for j in range(FC):
                    wt = wsmall.tile([128, DC, 128], F16, tag="w")
                    nc.sync.dma_start(out=wt, in_=w1_in[j])
                    ps = ps_mm.tile([128, L], F32, tag="mm")
                    for d in range(DC):
                        nc.tensor.matmul(ps, wt[:, d, :], xT[:, d, :],
                                         start=(d == 0), stop=(d == DC - 1))
                    nc.scalar.activation(hT[:, j, :], ps, AF.Gelu,
                                         bias=b1_t[:, j:j + 1])
                st2 = ln_stats_start()
                for j in range(DC):
                    wt = wbig.tile([128, FC, 128], F16, tag="w2")
                    nc.sync.dma_start(out=wt, in_=w2_in[j])
                    ps = ps_mm.tile([128, L], F32, tag="mm")
                    for f in range(FC):
                        nc.tensor.matmul(ps, wt[:, f, :], hT[:, f, :],
                                         start=(f == 0), stop=(f == FC - 1))
                    nc.vector.scalar_tensor_tensor(
                        xT[:, j, :], ps, b2_t[:, j:j + 1], xT[:, j, :],
                        op0=OP.add, op1=OP.add)
                    ln_stats_emit(st2, j)
                if "ln2" not in skips:
                    rm2 = ln_rstd(st2)
                    for d in range(DC):
                        ln_apply(rm2, d)

            xout_stage = glob.tile([128, DC, L], F32)
            nc.vector.tensor_copy(xout_stage, xT)
            nc.sync.dma_start(out=x_out[:], in_=xout_stage)
    nc.finalize()
    return nc


def _get_nc(n_layers):
    skips = tuple(s for s in os.environ.get("ATT_SKIPS", "").split(",") if s)
    debug = os.environ.get("ATT_DEBUG", "0") == "1"
    key = (n_layers, skips, debug)
    if key not in _CACHE:
        _CACHE[key] = _build_nc(n_layers, skips, debug)
    return _CACHE[key]


def _block4(w, rows, cols):
    # (rows*128, cols*128) weight -> [j, p, d, m] with
    # block[j, p, d, m] = w[j*128+m, d*128+p], laid out to match the SBUF
    # tile (128 partitions, cols chunks, 128) so the DMA is a plain copy.
    return np.ascontiguousarray(
        w.reshape(rows, 128, cols, 128).transpose(0, 3, 2, 1))


def kernel(states, actions, next_states, rewards, pos_embedding,
           embed_context_w, embed_context_b, embed_query_table,
           in_proj_w, in_proj_b, out_proj_w, out_proj_b,
           linear1_w, linear1_b, linear2_w, linear2_b,
           norm1_g, norm1_b, norm2_g, norm2_b, pred_w, pred_b,
           query_states, target_actions):
    global last_exec_time_ns
    from concourse.bass_utils import run_bass_kernel_spmd

    f32 = np.float32
    states = np.asarray(states, f32)
    actions = np.asarray(actions, f32)
    next_states = np.asarray(next_states, f32)
    rewards = np.asarray(rewards, f32)
    pos_embedding = np.asarray(pos_embedding, f32)
    in_proj_w = np.asarray(in_proj_w, f32)
    out_proj_w = np.asarray(out_proj_w, f32)
    linear1_w = np.asarray(linear1_w, f32)
    linear2_w = np.asarray(linear2_w, f32)

    # structural assumptions baked into the device graph
    assert np.all(np.asarray(in_proj_b)[2 * D:] == 0), "V bias unsupported"
    for g in (norm1_g, norm2_g):
        assert np.all(np.asarray(g) == 1), "LN gain unsupported"
    for b in (norm1_b, norm2_b):
        assert np.all(np.asarray(b) == 0), "LN bias unsupported"

    B = states.shape[0]
    assert B == NC

    # ---- host: input embedding ----
    qidx = np.asarray(query_states)[:, 0] * GRID + np.asarray(query_states)[:, 1]
    q_embed = np.asarray(embed_query_table, f32)[qidx][:, None, :]
    context = np.concatenate(
        [states, actions, rewards[..., None], next_states], axis=-1)
    ctx_embed = context @ np.asarray(embed_context_w, f32).T \
        + np.asarray(embed_context_b, f32)
    x0 = np.concatenate([ctx_embed, q_embed], axis=1) + pos_embedding[:, :L]

    # ---- shard + weight prep ----
    f16 = np.float16
    wqk = _block4(in_proj_w[:2 * D], 16, DC).astype(f16)
    wv = np.ascontiguousarray(in_proj_w[2 * D:].T.reshape(DC, 128, D)).astype(f16)
    wo = _block4(out_proj_w, DC, DC).astype(f16)
    w1 = _block4(linear1_w, FC, DC).astype(f16)
    w2 = _block4(linear2_w, DC, FC).astype(f16)
    bqk = np.ascontiguousarray(np.asarray(in_proj_b, f32)[:2 * D].reshape(16, 128).T)
    bo = np.ascontiguousarray(np.asarray(out_proj_b, f32).reshape(DC, 128).T)
    b1 = np.ascontiguousarray(np.asarray(linear1_b, f32).reshape(FC, 128).T)
    b2 = np.ascontiguousarray(np.asarray(linear2_b, f32).reshape(DC, 128).T)
    shared = dict(wqk=wqk, wv=wv, wo=wo, w1=w1, w2=w2,
                  bqk=bqk, bo=bo, b1=b1, b2=b2)
    in_maps = []
    for b in range(B):
        x0T = np.ascontiguousarray(
            x0[b].T.reshape(DC, 128, L).transpose(1, 0, 2)).astype(np.float16)
        in_maps.append(dict(x0T=x0T, **shared))

    n_layers = int(os.environ.get("ATT_LAYERS", N_LAYERS))
    nc = _get_nc(n_layers)
    trace = os.environ.get("TRN_KERNEL_TRACE", "0") == "1"
    res = run_bass_kernel_spmd(nc, in_maps, core_ids=list(range(NC)), trace=trace)
    last_exec_time_ns = res.exec_time_ns
    globals()["last_results"] = res

    # ---- host: gather + head ----
    attns = np.empty((n_layers, B, H, L, L), f32)
    xlast = np.empty((B, D), f32)
    for b in range(B):
        r = res.results[b]
        # attnT[lay, head, m, i, l]: token m_glob = i*128+m -> [lay, head, l, m_glob]
        a = r["attnT"].astype(f32)
        attns[:, b] = a.transpose(0, 1, 4, 3, 2).reshape(n_layers, H, L, L)
        xT_full = r["xoutT"].transpose(1, 0, 2).reshape(D, L)
        xlast[b] = xT_full[:, L - 1]

    logits = xlast @ np.asarray(pred_w, f32).T + np.asarray(pred_b, f32)
    mx = logits.max(-1, keepdims=True)
    lse = np.log(np.exp(logits - mx).sum(-1, keepdims=True)) + mx
    logp = logits - lse
    tgt = np.asarray(target_actions).astype(np.int64)
    nll = -np.take_along_axis(logp, tgt[:, None], axis=-1)[:, 0]
    loss = np.mean((1.0 - LABEL_SMOOTH) * nll - LABEL_SMOOTH * logp.mean(-1))
    acc = np.mean((np.argmax(logits, -1) == tgt).astype(f32))
    return np.float32(loss), np.float32(acc), attns
